# revision 2
# baseline (speedup 1.0000x reference)
"""Bidirectional AttGRU on 8 Trainium2 NeuronCores (Bass/Tile, SPMD).

Sharding: direction x2 (cores 0-3 forward, 4-7 backward) x batch/4
(16 batch rows per core). The backward direction is handled on the host by
time-reversing each backward core's context/att slices and feeding it the
backward weight set, so all 8 cores run the identical program (pure data
parallel, no collectives).

Per-core device program ("transposed world", all on-chip tensors [128, *]):
  phase 1: P^T = [Wr; W] @ c^T   (bf16 matmuls, f32 psum, + bias, bf16 store)
           written to DRAM scratch laid out [chunk][m-tile][128][CH*NB]
  phase 2: sequential scan over S=1024 steps; per step:
           yT = [Ur; U] @ hT      (72 bf16 128x128x16 MMs, f32 psum)
           r  = sigmoid(y_r + Pr[t])
           n  = r * y_u + Pw[t]   (+bU when nonzero)
           h' = h + (tanh(n) - h) * g[t]
  final h (f32, [128, 96]) is the only output.
"""

from contextlib import ExitStack

import numpy as np
import ml_dtypes

import concourse.bass as bass
import concourse.mybir as mybir
import concourse.tile as tile
from concourse import bacc
from concourse.bass_utils import run_bass_kernel_spmd

BF16 = ml_dtypes.bfloat16
F32 = mybir.dt.float32
BF = mybir.dt.bfloat16

H, S, NB, CH = 768, 1024, 16, 16
KT = H // 128           # 6  contraction tiles
MT = 2 * KT             # 12 output row tiles of [Ur;U] / [Wr;W]
GW = KT * NB            # 96 width of h-layout tiles
CHTOK = CH * NB         # 256 tokens per chunk
NCH = S // CH           # 64 chunks
NCORES = 8


def _build(ctx: ExitStack, tc: tile.TileContext, out_ap, ins: dict,
           skip_bu: bool = True):
    nc = tc.nc

    wpool = ctx.enter_context(tc.tile_pool(name="wpool", bufs=1))
    hpool = ctx.enter_context(tc.tile_pool(name="hpool", bufs=1))
    dram = ctx.enter_context(tc.tile_pool(name="dram", bufs=1, space="DRAM"))

    NW = MT * KT
    wproj_sb = wpool.tile([128, NW * 128], BF, tag="wproj")
    nc.sync.dma_start(wproj_sb[:].rearrange("p (n q) -> p n q", n=NW),
                      ins["wproj"].rearrange("n p q -> p n q"))
    wrec_sb = wpool.tile([128, NW * 128], BF, tag="wrec")
    nc.sync.dma_start(wrec_sb[:].rearrange("p (n q) -> p n q", n=NW),
                      ins["wrec"].rearrange("n p q -> p n q"))
    pbias_sb = wpool.tile([128, MT], F32, tag="pbias")
    nc.sync.dma_start(pbias_sb[:], ins["pbias"])
    bu_sb = None
    if not skip_bu:
        bu_sb = wpool.tile([128, GW], F32, tag="bu")
        nc.sync.dma_start(bu_sb[:], ins["buT"])

    h_a = hpool.tile([128, GW], F32, tag="h_a")
    h_b = hpool.tile([128, GW], F32, tag="h_b")
    hbf_a = hpool.tile([128, GW], BF, tag="hbf_a")
    hbf_b = hpool.tile([128, GW], BF, tag="hbf_b")
    nc.sync.dma_start(h_a[:], ins["h0T"])
    nc.vector.tensor_copy(hbf_a[:], h_a[:])

    scratch = dram.tile([NCH, MT, 128, CHTOK], BF)

    # ---- phase 1: projections ----
    ctxT = ins["ctxT"]
    with tc.tile_pool(name="cxp", bufs=2) as cxp, \
         tc.tile_pool(name="psp", bufs=4, space="PSUM") as psp, \
         tc.tile_pool(name="stp", bufs=4) as stp:
        for c in range(NCH):
            cx = cxp.tile([128, KT * CHTOK], BF, tag="cx")
            src = ctxT[:, c * CHTOK:(c + 1) * CHTOK].rearrange(
                "(k p) t -> p k t", p=128)
            nc.sync.dma_start(cx[:].rearrange("p (k t) -> p k t", k=KT), src)
            for m in range(MT):
                ps = psp.tile([128, CHTOK], F32, tag="ps")
                for k in range(KT):
                    nc.tensor.matmul(
                        ps[:],
                        wproj_sb[:, (m * KT + k) * 128:(m * KT + k + 1) * 128],
                        cx[:, k * CHTOK:(k + 1) * CHTOK],
                        start=(k == 0), stop=(k == KT - 1),
                    )
                st = stp.tile([128, CHTOK], BF, tag="st")
                nc.vector.tensor_scalar_add(st[:], ps[:], pbias_sb[:, m:m + 1])
                nc.sync.dma_start(scratch[c, m], st[:])

    # ---- phase 2: scan ----
    psr_pool = ctx.enter_context(tc.tile_pool(name="psy", bufs=3, space="PSUM"))
    chain = ctx.enter_context(tc.tile_pool(name="chain", bufs=3))
    g2 = ins["g96"]  # [NCH, 1, CH*GW] f32

    hs = [(h_a, hbf_a), (h_b, hbf_b)]

    def load(pipe, iv):
        s_sb = pipe.intermediate_tile([128, MT * CHTOK], BF)
        nc.sync.dma_start(s_sb[:].rearrange("p (m t) -> p m t", m=MT),
                          scratch[iv].rearrange("m p t -> p m t"))
        g_row = pipe.intermediate_tile([1, CH * GW], F32)
        nc.sync.dma_start(g_row[:], g2[iv])
        g_bc = pipe.intermediate_tile([128, CH * GW], F32)
        nc.gpsimd.partition_broadcast(g_bc[:], g_row[:])
        return (s_sb, g_bc)

    def compute(pipe, iv, tiles):
        s_sb, g_bc = tiles
        s4 = s_sb[:].rearrange("p (m c b) -> p m c b", m=MT, c=CH)
        for j in range(CH):
            h_prev, hbf_prev = hs[j % 2]
            h_next, hbf_next = hs[(j + 1) % 2]
            psr = psr_pool.tile([128, GW], F32, tag="psr")
            psu = psr_pool.tile([128, GW], F32, tag="psu")
            for m in range(MT):
                ps = psr if m < KT else psu
                mm = m if m < KT else m - KT
                for k in range(KT):
                    nc.tensor.matmul(
                        ps[:, mm * NB:(mm + 1) * NB],
                        wrec_sb[:, (m * KT + k) * 128:(m * KT + k + 1) * 128],
                        hbf_prev[:, k * NB:(k + 1) * NB],
                        start=(k == 0), stop=(k == KT - 1),
                    )
            s_r = s4[:, 0:KT, j, :]
            s_u = s4[:, KT:MT, j, :]
            psr3 = psr[:].rearrange("p (k b) -> p k b", k=KT)
            psu3 = psu[:].rearrange("p (k b) -> p k b", k=KT)

            pre_r = chain.tile([128, KT, NB], F32, tag="pre_r")
            nc.vector.tensor_tensor(pre_r[:], psr3, s_r, mybir.AluOpType.add)
            r = chain.tile([128, KT, NB], F32, tag="r")
            nc.scalar.activation(r[:], pre_r[:],
                                 mybir.ActivationFunctionType.Sigmoid)
            if bu_sb is not None:
                psu_b = chain.tile([128, KT, NB], F32, tag="psu_b")
                bu3 = bu_sb[:].rearrange("p (k b) -> p k b", k=KT)
                nc.vector.tensor_tensor(psu_b[:], psu3, bu3,
                                        mybir.AluOpType.add)
                u_in = psu_b[:]
            else:
                u_in = psu3
            m1 = chain.tile([128, KT, NB], F32, tag="m1")
            nc.vector.tensor_tensor(m1[:], r[:], u_in, mybir.AluOpType.mult)
            n = chain.tile([128, KT, NB], F32, tag="n")
            nc.vector.tensor_tensor(n[:], m1[:], s_u, mybir.AluOpType.add)
            htil = chain.tile([128, KT, NB], F32, tag="htil")
            nc.scalar.activation(htil[:], n[:],
                                 mybir.ActivationFunctionType.Tanh)
            d = chain.tile([128, KT, NB], F32, tag="d")
            h3_prev = h_prev[:].rearrange("p (k b) -> p k b", k=KT)
            nc.vector.tensor_tensor(d[:], htil[:], h3_prev,
                                    mybir.AluOpType.subtract)
            e = chain.tile([128, KT, NB], F32, tag="e")
            g3 = g_bc[:, j * GW:(j + 1) * GW].rearrange("p (k b) -> p k b", k=KT)
            nc.vector.tensor_tensor(e[:], d[:], g3, mybir.AluOpType.mult)
            h3_next = h_next[:].rearrange("p (k b) -> p k b", k=KT)
            nc.vector.tensor_tensor(h3_next, h3_prev, e[:], mybir.AluOpType.add)
            nc.vector.tensor_copy(hbf_next[:], h_next[:])

    tc.For_i_pipelined(
        [load, compute], 0, NCH, unroll=2,
        hint_engines=(mybir.EngineType.PE,),
        name="scan",
    )

    nc.sync.dma_start(out_ap, h_a[:])


# ---------------- host side ----------------

def _host_prep_core(context, init_hidden, att_score, w, dir_bwd, q):
    b0 = q * NB
    ctx_q = context[b0:b0 + NB]
    att_q = att_score[b0:b0 + NB]
    h0_q = init_hidden[b0:b0 + NB]
    if dir_bwd:
        ctx_q = ctx_q[:, ::-1]
        att_q = att_q[:, ::-1]

    ctxT = np.ascontiguousarray(
        ctx_q.transpose(2, 1, 0).reshape(H, S * NB)).astype(BF16)

    def tiles_of(Wcat):
        t = np.empty((MT * KT, 128, 128), np.float32)
        for m in range(MT):
            for k in range(KT):
                t[m * KT + k] = \
                    Wcat[128 * m:128 * (m + 1), 128 * k:128 * (k + 1)].T
        return t.astype(BF16)

    wrec = tiles_of(np.concatenate([w["Ur"], w["U"]], 0))
    wproj = tiles_of(np.concatenate([w["Wr"], w["W"]], 0))
    bias_cat = np.concatenate([w["bWr"] + w["bUr"], w["bW"]])
    pbias = np.ascontiguousarray(bias_cat.reshape(MT, 128).T).astype(np.float32)

    g96 = np.tile(att_q.T, (1, KT)).reshape(NCH, 1, CH * GW).astype(np.float32)
    h0T = np.ascontiguousarray(
        h0_q.T.reshape(KT, 128, NB).transpose(1, 0, 2).reshape(128, GW)
    ).astype(np.float32)
    buT = np.ascontiguousarray(
        np.broadcast_to(w["bU"].reshape(KT, 128).T[:, :, None], (128, KT, NB))
    ).reshape(128, GW).astype(np.float32)
    return {"ctxT": ctxT, "wproj": wproj, "wrec": wrec, "pbias": pbias,
            "g96": g96, "h0T": h0T, "buT": buT}


def _host_post_core(o):
    return np.ascontiguousarray(
        o.reshape(128, KT, NB).transpose(2, 1, 0).reshape(NB, H))


_IN_SPECS = {
    "ctxT": ((H, S * NB), BF),
    "wproj": ((MT * KT, 128, 128), BF),
    "wrec": ((MT * KT, 128, 128), BF),
    "pbias": ((128, MT), F32),
    "g96": ((NCH, 1, CH * GW), F32),
    "h0T": ((128, GW), F32),
    "buT": ((128, GW), F32),
}


def _build_graph(skip_bu):
    nc = bacc.Bacc("TRN2", target_bir_lowering=False, debug=False,
                   enable_asserts=False, num_devices=NCORES)
    ins = {}
    for name, (shape, dt) in _IN_SPECS.items():
        if name == "buT" and skip_bu:
            continue
        ins[name] = nc.dram_tensor(name, shape, dt, kind="ExternalInput").ap()
    out_ap = nc.dram_tensor("out", (128, GW), F32, kind="ExternalOutput").ap()
    with tile.TileContext(nc) as tc:
        with ExitStack() as ctx:
            _build(ctx, tc, out_ap, ins, skip_bu=skip_bu)
    nc.compile()
    return nc


def run(inputs, trace=False, trace_kwargs=None):
    inputs = {k: np.asarray(v) for k, v in inputs.items()}
    context = inputs["context"].astype(np.float32, copy=False)
    init_hidden = inputs["init_hidden"].astype(np.float32, copy=False)
    att_score = inputs["att_score"].astype(np.float32, copy=False)

    wsets = {}
    for d in ("f", "b"):
        wsets[d] = {k: inputs[f"{k}_{d}"].astype(np.float32, copy=False)
                    for k in ("Wr", "Ur", "W", "U", "bWr", "bUr", "bW", "bU")}
    skip_bu = bool(np.all(wsets["f"]["bU"] == 0) and np.all(wsets["b"]["bU"] == 0))

    nc = _build_graph(skip_bu)

    in_maps = []
    for core in range(NCORES):
        dir_bwd = core >= 4
        q = core % 4
        m = _host_prep_core(context, init_hidden, att_score,
                            wsets["b" if dir_bwd else "f"], dir_bwd, q)
        if skip_bu:
            m.pop("buT")
        in_maps.append(m)

    res = run_bass_kernel_spmd(
        nc, in_maps, core_ids=list(range(NCORES)),
        trace=trace, **(trace_kwargs or {}))

    out = np.empty((64, 1, 2 * H), np.float32)
    for core in range(NCORES):
        h_q = _host_post_core(np.asarray(res.results[core]["out"]))
        q = core % 4
        if core < 4:
            out[q * NB:(q + 1) * NB, 0, :H] = h_q
        else:
            out[q * NB:(q + 1) * NB, 0, H:] = h_q
    return out, res


def kernel(**inputs) -> np.ndarray:
    out, _ = run(inputs, trace=False)
    return out


# revision 23
# speedup vs baseline: 1.2826x; 1.2826x over previous
"""Bidirectional AttGRU on 8 Trainium2 NeuronCores (Bass/Tile, SPMD).

Sharding: direction x2 (cores 0-3 forward, 4-7 backward) x batch/4
(16 batch rows per core). The backward direction is handled on the host by
time-reversing each backward core's context/att slices and feeding it the
backward weight set, so all 8 cores run the identical program (pure data
parallel, no collectives).

Per-core device program ("transposed world", all on-chip tensors [128, *]):
sequence is processed in chunks of CH=8 steps; the context projections
P^T = [Wr; W] @ c^T for chunk c+1 are computed into PSUM (bank pair B)
while the scan consumes chunk c from bank pair A. The r-side recurrent
matmuls accumulate Ur@h directly on top of the projection PSUM, so
  r = sigmoid(psum)                 (one ACT op, no pre-add)
  n = (r * psu) + Pw_psum           (two DVE ops)
  h' = tanh(n)*g + (1-g)*h          (b=(1-g)*h precomputed off-path)
h stays f32; recurrent matmuls run with bf16 weights x float32r h.
"""

from contextlib import ExitStack

import numpy as np
import ml_dtypes

import concourse.bass as bass
import concourse.mybir as mybir
import concourse.tile as tile
from concourse import bacc
from concourse.bass_utils import run_bass_kernel_spmd

BF16 = ml_dtypes.bfloat16
F32 = mybir.dt.float32
F32R = mybir.dt.float32r
BF = mybir.dt.bfloat16
ALU = mybir.AluOpType
AF = mybir.ActivationFunctionType

H, S, NB, CH = 768, 1024, 16, 8
KT = H // 128            # 6   contraction tiles
MT = 2 * KT              # 12  row tiles of [Wr; W] / [Ur; U]
GW = KT * NB             # 96  h-layout width
CHTOK = CH * NB          # 128 tokens per chunk
NCH = S // CH            # 128 chunks
NQUAD = NCH // 4         # 32  loop iterations (4 chunks per body)
NW = MT * KT             # 72  weight tiles
NCORES = 8


def _build(ctx: ExitStack, tc: tile.TileContext, out_ap, ins: dict,
           zero_bias: bool):
    nc = tc.nc

    wpool = ctx.enter_context(tc.tile_pool(name="wpool", bufs=1))
    hpool = ctx.enter_context(tc.tile_pool(name="hpool", bufs=1))
    gpool = ctx.enter_context(tc.tile_pool(name="gpool", bufs=1))
    cxpool = ctx.enter_context(tc.tile_pool(name="cxpool", bufs=1))
    ppool = ctx.enter_context(tc.tile_pool(name="ppool", bufs=1, space="PSUM"))
    upool = ctx.enter_context(tc.tile_pool(name="upool", bufs=1, space="PSUM"))
    chain = ctx.enter_context(tc.tile_pool(name="chain", bufs=3))

    # ---- weights / constants ----
    wproj_sb = wpool.tile([128, NW * 128], BF, tag="wproj")
    nc.sync.dma_start(wproj_sb[:].rearrange("p (n q) -> p n q", n=NW),
                      ins["wproj"].rearrange("n p q -> p n q"))
    wrec_sb = wpool.tile([128, NW * 128], BF, tag="wrec")
    nc.sync.dma_start(wrec_sb[:].rearrange("p (n q) -> p n q", n=NW),
                      ins["wrec"].rearrange("n p q -> p n q"))

    bias_tiles = {}
    if not zero_bias:
        for nm in ("rbias", "wbias", "bu"):
            t = wpool.tile([128, GW], F32, tag=nm)
            nc.sync.dma_start(t[:], ins[nm])
            bias_tiles[nm] = t

    h_t = [hpool.tile([128, GW], F32, tag=f"h_{i}", name=f"h_{i}")
           for i in range(2)]
    hbf_t = [hpool.tile([128, GW], BF, tag=f"hbf_{i}", name=f"hbf_{i}")
             for i in range(2)]
    b_t = [hpool.tile([128, KT, NB], F32, tag=f"b_{i}", name=f"b_{i}")
           for i in range(2)]
    nc.sync.dma_start(h_t[0][:], ins["h0T"])
    nc.vector.tensor_copy(hbf_t[0][:], h_t[0][:])

    # per-parity buffers: context chunks, g / (1-g) broadcasts, proj PSUM
    cx = [cxpool.tile([128, KT * CHTOK], BF, tag=f"cx{p}", name=f"cx{p}")
          for p in range(2)]
    g_bc = [gpool.tile([128, CH * GW], F32, tag=f"g{p}", name=f"g{p}")
            for p in range(2)]
    og_bc = [gpool.tile([128, CH * GW], F32, tag=f"og{p}", name=f"og{p}")
             for p in range(2)]
    proj = [ppool.tile([128, MT * CHTOK], F32, tag=f"proj{p}", name=f"proj{p}")
            for p in range(2)]
    projr = [gpool.tile([128, KT * CHTOK], F32, tag=f"projr{p}", name=f"projr{p}")
             for p in range(2)]

    def projr_copy(par, piece):
        # r-half of the projection PSUM -> SBUF (off the critical path);
        # needed because the per-step pre-add already reads one PSUM operand.
        # Done in thirds right after the corresponding m-planes finish, so no
        # single 800ns copy ever sits in front of a critical ACT op.
        w = 2 * CHTOK
        nc.scalar.copy(projr[par][:, piece * w:(piece + 1) * w],
                       proj[par][:, piece * w:(piece + 1) * w])

    def load_chunk(par, ctx_src, g_src, og_src):
        nc.sync.dma_start(cx[par][:].rearrange("p (k t) -> p k t", k=KT),
                          ctx_src)
        nc.sync.dma_start(g_bc[par][:], g_src.to_broadcast((128, CH * GW)))
        nc.sync.dma_start(og_bc[par][:], og_src.to_broadcast((128, CH * GW)))

    def proj_mms(par, m):
        p4 = proj[par][:].rearrange("p (m t) -> p m t", m=MT)
        for k in range(KT):
            nc.tensor.matmul(
                p4[:, m, :],
                wproj_sb[:, (m * KT + k) * 128:(m * KT + k + 1) * 128],
                cx[par][:, k * CHTOK:(k + 1) * CHTOK],
                start=(k == 0), stop=(k == KT - 1),
            )

    def proj_bias(par):
        if zero_bias:
            return
        p4 = proj[par][:].rearrange("p (m c b) -> p m c b", m=MT, c=CH)
        pr4 = projr[par][:].rearrange("p (m c b) -> p m c b", m=KT, c=CH)
        rb = bias_tiles["rbias"][:].rearrange("p (k b) -> p k b", k=KT)
        wb = bias_tiles["wbias"][:].rearrange("p (k b) -> p k b", k=KT)
        for j in range(CH):
            nc.vector.tensor_tensor(pr4[:, :, j, :], pr4[:, :, j, :],
                                    rb, ALU.add)
            nc.vector.tensor_tensor(p4[:, KT:MT, j, :], p4[:, KT:MT, j, :],
                                    wb, ALU.add)

    def scan_step(par, j, s):
        """step s (global), chunk parity par, step-in-chunk j."""
        h_prev = h_t[s % 2]
        h_next = h_t[(s + 1) % 2]
        b_cur = b_t[s % 2]
        b_nxt = b_t[(s + 1) % 2]
        p4 = proj[par][:].rearrange("p (m t) -> p m t", m=MT)

        psr = upool.tile([128, GW], F32, tag="psr")
        psu = upool.tile([128, GW], F32, tag="psu")
        hbf_prev = hbf_t[s % 2]
        rhs_of = lambda k: hbf_prev[:, k * NB:(k + 1) * NB]
        for m in range(KT):
            for k in range(KT):
                nc.tensor.matmul(
                    psr[:, m * NB:(m + 1) * NB],
                    wrec_sb[:, (m * KT + k) * 128:(m * KT + k + 1) * 128],
                    rhs_of(k),
                    start=(k == 0), stop=(k == KT - 1),
                )
        for m in range(KT):
            for k in range(KT):
                nc.tensor.matmul(
                    psu[:, m * NB:(m + 1) * NB],
                    wrec_sb[:, ((m + KT) * KT + k) * 128:((m + KT) * KT + k + 1) * 128],
                    rhs_of(k),
                    start=(k == 0), stop=(k == KT - 1),
                )

        p5 = proj[par][:].rearrange("p (m c b) -> p m c b", m=MT, c=CH)
        r_in = projr[par][:].rearrange(
            "p (m c b) -> p m c b", m=KT, c=CH)[:, :, j, :]
        u_pr = p5[:, KT:MT, j, :]
        psu3 = psu[:].rearrange("p (k b) -> p k b", k=KT)
        g3 = g_bc[par][:, j * GW:(j + 1) * GW].rearrange("p (k b) -> p k b", k=KT)

        r = chain.tile([128, KT, NB], F32, tag="r")
        prer = chain.tile([128, KT, NB], F32, tag="prer")
        nc.vector.tensor_tensor(
            prer[:], psr[:].rearrange("p (k b) -> p k b", k=KT), r_in,
            ALU.add)
        nc.scalar.activation(r[:], prer[:], AF.Sigmoid)
        if not zero_bias:
            ub = chain.tile([128, KT, NB], F32, tag="ub")
            bu3 = bias_tiles["bu"][:].rearrange("p (k b) -> p k b", k=KT)
            nc.vector.tensor_tensor(ub[:], psu3, bu3, ALU.add)
            u_in = ub[:]
        else:
            u_in = psu3
        m1 = chain.tile([128, KT, NB], F32, tag="m1")
        nc.vector.tensor_tensor(m1[:], r[:], u_in, ALU.mult)
        n = chain.tile([128, KT, NB], F32, tag="n")
        nc.vector.tensor_tensor(n[:], m1[:], u_pr, ALU.add)
        htil = chain.tile([128, KT, NB], F32, tag="htil")
        nc.scalar.activation(htil[:], n[:], AF.Tanh)
        h3_next = h_next[:].rearrange("p (k b) -> p k b", k=KT)
        hbf_next = hbf_t[(s + 1) % 2]
        a = chain.tile([128, KT, NB], F32, tag="a")
        nc.vector.tensor_tensor(a[:], htil[:], g3, ALU.mult)
        nc.vector.tensor_tensor(h3_next, a[:], b_cur[:], ALU.add)
        nc.gpsimd.tensor_copy(hbf_next[:], h_next[:])

        # off-critical-path: b for step s+1 = (1-g_{s+1}) * h_next
        if j + 1 < CH:
            og_nxt = og_bc[par][:, (j + 1) * GW:(j + 2) * GW]
        else:
            og_nxt = og_bc[1 - par][:, 0:GW]
        nc.gpsimd.tensor_tensor(b_nxt[:], h3_next,
                                 og_nxt.rearrange("p (k b) -> p k b", k=KT),
                                 ALU.mult)

    # ---- prologue: chunks 0 and 1 staged, proj(0) in parity A ----
    load_chunk(0, ins["ctx_first"][0].rearrange("p (k t) -> p k t", k=KT),
               ins["g_first"][0], ins["og_first"][0])
    load_chunk(1, ins["ctx_first"][1].rearrange("p (k t) -> p k t", k=KT),
               ins["g_first"][1], ins["og_first"][1])
    for m in range(MT):
        proj_mms(0, m)
    for piece in range(3):
        projr_copy(0, piece)
    proj_bias(0)
    # b for step 0
    nc.vector.tensor_tensor(
        b_t[0][:],
        h_t[0][:].rearrange("p (k b) -> p k b", k=KT),
        og_bc[0][:, 0:GW].rearrange("p (k b) -> p k b", k=KT),
        ALU.mult)

    # ---- main loop: body handles chunk pair (2i, 2i+1) ----
    ctx_pairs = ins["ctx_pairs"]
    g_pairs = ins["g_pairs"]
    og_pairs = ins["og_pairs"]

    with tc.For_i(0, NQUAD, 1, hint_engines=(mybir.EngineType.PE,),
                  name="scan") as iv:
        # quad row c = body-chunk c+2; cx[0] first load feeds proj during chunk 1
        nc.sync.dma_start(cx[0][:].rearrange("p (k t) -> p k t", k=KT),
                          ctx_pairs[iv, 0].rearrange("p (k t) -> p k t", k=KT))
        for c4 in range(4):
            par = c4 % 2
            for j in range(CH):
                scan_step(par, j, c4 * CH + j)
                if j < 6:
                    proj_mms(1 - par, 2 * j)
                    proj_mms(1 - par, 2 * j + 1)
                if j < 3:
                    projr_copy(1 - par, j)
            proj_bias(1 - par)
            # prefetches unlocked by this chunk's completion
            nc.sync.dma_start(g_bc[par][:],
                              g_pairs[iv, c4].to_broadcast((128, CH * GW)))
            nc.sync.dma_start(og_bc[par][:],
                              og_pairs[iv, c4].to_broadcast((128, CH * GW)))
            if c4 < 3:
                nc.sync.dma_start(
                    cx[1 - par][:].rearrange("p (k t) -> p k t", k=KT),
                    ctx_pairs[iv, c4 + 1].rearrange("p (k t) -> p k t", k=KT))

    nc.sync.dma_start(out_ap, h_t[0][:])


# ---------------- host side ----------------

def _host_prep_core(context, init_hidden, att_score, w, dir_bwd, q):
    b0 = q * NB
    ctx_q = context[b0:b0 + NB]
    att_q = att_score[b0:b0 + NB]
    h0_q = init_hidden[b0:b0 + NB]
    if dir_bwd:
        ctx_q = ctx_q[:, ::-1]
        att_q = att_q[:, ::-1]

    # context chunks: [NCH, 128, KT*CHTOK]; chunk c col (k, t) row p =
    # c[batch t%NB, step c*CH + t//NB, 128k+p]
    ctxT = np.ascontiguousarray(
        ctx_q.transpose(2, 1, 0).reshape(H, S * NB)).astype(BF16)
    chunks = np.ascontiguousarray(
        ctxT.reshape(KT, 128, NCH, CHTOK).transpose(2, 1, 0, 3)
    ).reshape(NCH, 128, KT * CHTOK)
    pad = np.zeros((4 * NQUAD + 2 - NCH, 128, KT * CHTOK), BF16)
    chunks = np.concatenate([chunks, pad], 0)           # NCH+2
    ctx_first = np.ascontiguousarray(chunks[:2])
    ctx_pairs = np.ascontiguousarray(chunks[2:].reshape(NQUAD, 4, 128, KT * CHTOK))

    def tiles_of(Wcat, dt):
        t = np.empty((NW, 128, 128), np.float32)
        for m in range(MT):
            for k in range(KT):
                t[m * KT + k] = \
                    Wcat[128 * m:128 * (m + 1), 128 * k:128 * (k + 1)].T
        return t.astype(dt)

    wrec = tiles_of(np.concatenate([w["Ur"], w["U"]], 0), BF16)
    wproj = tiles_of(np.concatenate([w["Wr"], w["W"]], 0), BF16)

    # g/(1-g) rows per chunk: [NCH, 1, CH*GW]; col (c_in_chunk j, k, b) -> g[step, b]
    g96 = np.tile(att_q.T, (1, KT)).reshape(NCH, 1, CH * GW).astype(np.float32)
    og96 = np.tile(1.0 - att_q.T, (1, KT)).reshape(NCH, 1, CH * GW).astype(np.float32)
    gpad = np.zeros((4 * NQUAD + 2 - NCH, 1, CH * GW), np.float32)
    g96 = np.concatenate([g96, gpad], 0)
    og96 = np.concatenate([og96, gpad], 0)
    g_first = np.ascontiguousarray(g96[:2])
    g_pairs = np.ascontiguousarray(g96[2:].reshape(NQUAD, 4, 1, CH * GW))
    og_first = np.ascontiguousarray(og96[:2])
    og_pairs = np.ascontiguousarray(og96[2:].reshape(NQUAD, 4, 1, CH * GW))

    h0T = np.ascontiguousarray(
        h0_q.T.reshape(KT, 128, NB).transpose(1, 0, 2).reshape(128, GW)
    ).astype(np.float32)

    def bcast_t(v):   # [H] -> [128, GW] in h-layout
        return np.ascontiguousarray(
            np.broadcast_to(v.reshape(KT, 128).T[:, :, None], (128, KT, NB))
        ).reshape(128, GW).astype(np.float32)

    return {"ctx_first": ctx_first, "ctx_pairs": ctx_pairs,
            "wproj": wproj, "wrec": wrec,
            "g_first": g_first, "g_pairs": g_pairs,
            "og_first": og_first, "og_pairs": og_pairs,
            "h0T": h0T,
            "rbias": bcast_t(w["bWr"] + w["bUr"]),
            "wbias": bcast_t(w["bW"]),
            "bu": bcast_t(w["bU"])}


def _host_post_core(o):
    return np.ascontiguousarray(
        o.reshape(128, KT, NB).transpose(2, 1, 0).reshape(NB, H))


def _in_specs():
    return {
        "ctx_first": ((2, 128, KT * CHTOK), BF),
        "ctx_pairs": ((NQUAD, 4, 128, KT * CHTOK), BF),
        "wproj": ((NW, 128, 128), BF),
        "wrec": ((NW, 128, 128), BF),
        "g_first": ((2, 1, CH * GW), F32),
        "g_pairs": ((NQUAD, 4, 1, CH * GW), F32),
        "og_first": ((2, 1, CH * GW), F32),
        "og_pairs": ((NQUAD, 4, 1, CH * GW), F32),
        "h0T": ((128, GW), F32),
        "rbias": ((128, GW), F32),
        "wbias": ((128, GW), F32),
        "bu": ((128, GW), F32),
    }


_BIAS_NAMES = ("rbias", "wbias", "bu")


def _build_graph(zero_bias):
    nc = bacc.Bacc("TRN2", target_bir_lowering=False, debug=False,
                   enable_asserts=False, num_devices=NCORES)
    ins = {}
    for name, (shape, dt) in _in_specs().items():
        if zero_bias and name in _BIAS_NAMES:
            continue
        ins[name] = nc.dram_tensor(name, shape, dt, kind="ExternalInput").ap()
    out_ap = nc.dram_tensor("out", (128, GW), F32, kind="ExternalOutput").ap()
    with tile.TileContext(nc) as tc:
        with ExitStack() as ctx:
            _build(ctx, tc, out_ap, ins, zero_bias)
    nc.compile()
    return nc


def run(inputs, trace=False, trace_kwargs=None):
    inputs = {k: np.asarray(v) for k, v in inputs.items()}
    context = inputs["context"].astype(np.float32, copy=False)
    init_hidden = inputs["init_hidden"].astype(np.float32, copy=False)
    att_score = inputs["att_score"].astype(np.float32, copy=False)

    wsets = {}
    for d in ("f", "b"):
        wsets[d] = {k: inputs[f"{k}_{d}"].astype(np.float32, copy=False)
                    for k in ("Wr", "Ur", "W", "U", "bWr", "bUr", "bW", "bU")}
    zero_bias = all(
        np.all(wsets[d][b] == 0)
        for d in ("f", "b") for b in ("bWr", "bUr", "bW", "bU"))

    nc = _build_graph(zero_bias)

    in_maps = []
    for core in range(NCORES):
        dir_bwd = core >= 4
        q = core % 4
        m = _host_prep_core(context, init_hidden, att_score,
                            wsets["b" if dir_bwd else "f"], dir_bwd, q)
        if zero_bias:
            for b in _BIAS_NAMES:
                m.pop(b)
        in_maps.append(m)

    res = run_bass_kernel_spmd(
        nc, in_maps, core_ids=list(range(NCORES)),
        trace=trace, **(trace_kwargs or {}))

    out = np.empty((64, 1, 2 * H), np.float32)
    for core in range(NCORES):
        h_q = _host_post_core(np.asarray(res.results[core]["out"]))
        q = core % 4
        if core < 4:
            out[q * NB:(q + 1) * NB, 0, :H] = h_q
        else:
            out[q * NB:(q + 1) * NB, 0, H:] = h_q
    return out, res


def kernel(**inputs) -> np.ndarray:
    out, _ = run(inputs, trace=False)
    return out
